# revision 22
# baseline (speedup 1.0000x reference)
"""Trainium2 Bass kernel for causal multi-head attention (fp8-DoubleRow v2).

Problem: B=4, S=2048, D=1024, H=16 heads, Dh=64, fp32, causal mask.
Sharding: 8 cores = 4 batches x 2 head-groups (8 heads each). No
collectives: each core produces a partial output projection y_T
[1024, 2048] (bf16) for its batch; the host sums the two head-group
partials per batch in fp32 and adds the output bias.

v2 moves the projection and score matmuls to fp8e4 DoubleRow (0.5
cycles/row, 2x128 contraction per instruction) with residual
decompositions that keep the end-to-end error at ~7e-3:
  - Q/K/V projections: x = x1+x2 and W = W1+W2 (both fp8, quantized on
    the host at power-of-2 scales x*4, W*128); products W1x1 + W1x2 +
    W2x1 accumulate in psum (the dropped W2x2 term is O(fp8^2)).
    3 sets x 4 mt-pair DR matmuls = 3072 cycles/group vs 4096 bf16.
  - scores: S_raw = (K1+K2)^T Q~ in ONE DR matmul per (head, k-tile):
    group 0 = K1, group 1 = K2 (the fp8 residual of 8K, computed on
    device), rhs = Q~ (fp8 of 8Q) broadcast stride-0 across the two
    groups. K side gets fp8^2 accuracy, Q side single-fp8.
  - attention@V and the output projection stay bf16: fp8 attention
    probs measured 1.7e-2 end-to-end (uniform-attention probs carry
    the full signal), over the 2e-2 budget.
  - exp on ACT with scale SCALE/64 (descales the 8Q*8K raw scores);
    v_sb holds 512*V bf16 so the V bias-add fuses in one DVE op; the
    1/512 rides the reciprocal-broadcast sel matrix (entries 1/512).
  - the two per-pair denominator reciprocal broadcasts merge into ONE
    matmul: sel [2,128] lhsT ((1/512)*indicator halves), rhs [2,512] =
    (1/denom_even, 1/denom_odd) rows.
  - all projection groups (proj0-2) are deferred as PE filler into
    attn3, which is otherwise exp(ACT)-bound: attn3's ACT load (~59us)
    exceeds its own PE work (~38us). y tile copies run on the idle
    gpsimd(Pool) engine.
"""

import numpy as np
import ml_dtypes

import concourse.tile as tile
from concourse import bacc, mybir
from concourse.bass_utils import run_bass_kernel_spmd

B = 4
S = 2048
D = 1024
H = 16
DH = 64
NCORES = 8
HPC = 8  # heads per core
C = HPC * DH  # 512 local channels per core
QB = 512  # q-block (matmul moving free dim)
NQB = S // QB  # 4
NKT = S // 128  # 16 k-tiles
SCALE = 1.0 / float(np.sqrt(DH))

F32 = mybir.dt.float32
F32R = mybir.dt.float32r
BF16 = mybir.dt.bfloat16
FP8 = mybir.dt.float8e4
DR = mybir.MatmulPerfMode.DoubleRow
AF = mybir.ActivationFunctionType
ALU = mybir.AluOpType
E4 = ml_dtypes.float8_e4m3

# 3 DR product sets (W1x1+W1x2+W2x1) = both-side residual; 2 sets
# (W1x1+W1x2) would be x-residual only (faster, ~8e-3 more error).
QK_SETS = 3


def build_nc():
    nc = bacc.Bacc("TRN2", target_bir_lowering=False, debug=False)
    regions = []
    nc._regions = regions

    def region(name):
        # record the number of PE matmuls emitted so far: matmul ordinal is
        # stable across compile-time instruction insertion, so the analyzer
        # can zip time-sorted Matmult slices against it
        nmm = sum(
            1 for v in nc.inst_map.values()
            if type(v).__name__ == "InstMatmult"
        )
        regions.append((name, nmm))

    x1t = nc.dram_tensor("x1t", [D, S], FP8, kind="ExternalInput").ap()
    x2t = nc.dram_tensor("x2t", [D, S], FP8, kind="ExternalInput").ap()
    wq1t = nc.dram_tensor("wq1t", [D, C], FP8, kind="ExternalInput").ap()
    wq2t = nc.dram_tensor("wq2t", [D, C], FP8, kind="ExternalInput").ap()
    wk1t = nc.dram_tensor("wk1t", [D, C], FP8, kind="ExternalInput").ap()
    wk2t = nc.dram_tensor("wk2t", [D, C], FP8, kind="ExternalInput").ap()
    wv1t = nc.dram_tensor("wv1t", [D, C], FP8, kind="ExternalInput").ap()
    wv2t = nc.dram_tensor("wv2t", [D, C], FP8, kind="ExternalInput").ap()
    wot = nc.dram_tensor("wot", [C, D], BF16, kind="ExternalInput").ap()
    bq_d = nc.dram_tensor("bq8", [128, C // 128], F32, kind="ExternalInput").ap()
    bk_d = nc.dram_tensor("bk8", [128, C // 128], F32, kind="ExternalInput").ap()
    bvb_d = nc.dram_tensor("bvb", [128, C], F32, kind="ExternalInput").ap()
    sel_d = nc.dram_tensor("sel", [1, 128], F32R, kind="ExternalInput").ap()
    yt = nc.dram_tensor("yt", [D, S], BF16, kind="ExternalOutput").ap()

    x1_r = x1t.rearrange("(mt p) s -> p mt s", p=128)
    x2_r = x2t.rearrange("(mt p) s -> p mt s", p=128)

    with tile.TileContext(nc) as tc:
        with (
            tc.tile_pool(name="singles", bufs=1) as singles,
            tc.tile_pool(name="x1p", bufs=2) as x1p,
            tc.tile_pool(name="x2p", bufs=2) as x2p,
            tc.tile_pool(name="qtp", bufs=2) as qtp,
            tc.tile_pool(name="s8p", bufs=2) as s8p,
            tc.tile_pool(name="aop", bufs=4) as aop,
            tc.tile_pool(name="pp", bufs=3) as pp,
            tc.tile_pool(name="rp", bufs=2) as rp,
            tc.tile_pool(name="yp", bufs=4) as yp,
            tc.tile_pool(name="bcp", bufs=2) as bcp,
            tc.tile_pool(name="ps_mm", bufs=2, space="PSUM") as ps_mm,
            tc.tile_pool(name="ps_s", bufs=2, space="PSUM") as ps_s_pool,
            tc.tile_pool(name="ps_o", bufs=2, space="PSUM") as ps_o_pool,
        ):
            # ---- persistent tiles -------------------------------------
            w_q1 = singles.tile([128, 8, C], FP8, tag="w_q1")
            w_q2 = singles.tile([128, 8, C], FP8, tag="w_q2")
            w_k1 = singles.tile([128, 8, C], FP8, tag="w_k1")
            w_k2 = singles.tile([128, 8, C], FP8, tag="w_k2")
            w_v1 = singles.tile([128, 8, C], FP8, tag="w_v1")
            w_v2 = singles.tile([128, 8, C], FP8, tag="w_v2")
            w_o = singles.tile([128, 4, D], BF16, tag="w_o")
            bq_sb = singles.tile([128, C // 128], F32, tag="bq")
            bk_sb = singles.tile([128, C // 128], F32, tag="bk")
            bvb_sb = singles.tile([128, C], F32, tag="bvb")
            sel_sb = singles.tile([1, 128], F32R, tag="sel")
            # K~ = (K1, K2): res dim is the DR group axis
            kt8 = singles.tile([128, 4, 2, S], FP8, tag="kt8")
            v_sb = singles.tile([128, NKT, HPC, DH + 1], BF16, tag="v")
            masks = singles.tile([128, 2, QB], BF16, tag="masks")

            # one whole-tensor dma_start each: HWDGE dispatch is ~625ns
            # serialized per instruction, so small per-mt chunking wastes
            # tens of us; a full [128,8,C] fp8 tensor moves in ~1.5us
            x1_cur = x1p.tile([128, 8, QB], FP8, tag="x1")
            x2_cur = x2p.tile([128, 8, QB], FP8, tag="x2")
            wq1_r = wq1t.rearrange("(mt p) j -> p mt j", p=128)
            wq2_r = wq2t.rearrange("(mt p) j -> p mt j", p=128)
            wk1_r = wk1t.rearrange("(mt p) j -> p mt j", p=128)
            wk2_r = wk2t.rearrange("(mt p) j -> p mt j", p=128)
            wv1_r = wv1t.rearrange("(mt p) j -> p mt j", p=128)
            wv2_r = wv2t.rearrange("(mt p) j -> p mt j", p=128)
            nc.sync.dma_start(x1_cur, x1_r[:, :, 0:QB])
            nc.sync.dma_start(w_q1, wq1_r)
            nc.sync.dma_start(x2_cur, x2_r[:, :, 0:QB])
            nc.sync.dma_start(w_q2, wq2_r)
            nc.sync.dma_start(w_k1, wk1_r)
            nc.sync.dma_start(w_k2, wk2_r)
            nc.sync.dma_start(w_v1, wv1_r)
            nc.sync.dma_start(w_v2, wv2_r)
            # small/constant inputs ride the idle gpsimd (SWDGE) queue
            nc.gpsimd.dma_start(bq_sb, bq_d)
            nc.gpsimd.dma_start(bk_sb, bk_d)
            nc.gpsimd.dma_start(bvb_sb, bvb_d)
            nc.gpsimd.dma_start(sel_sb, sel_d)

            # ones column (65th) of every per-head V block
            nc.vector.memset(v_sb[:, :, :, DH : DH + 1], 1.0)
            # mask tile; only the [128:256] slice of row 0 is used — in
            # band-local coordinates it is the f>=p triangle that every
            # diagonal tile needs
            nc.vector.memset(masks, 1.0)
            # warm-up matmuls on the freshly-memset mask tile: they depend
            # only on the early DVE memset, so they execute during the
            # initial DMA wait and keep the PE activity window warm
            for _ in range(8):
                ps_w = ps_mm.tile([128, QB], F32, tag="mm")
                nc.tensor.matmul(
                    ps_w, masks[:, 0, 0:128], masks[:, 1, :], start=True, stop=True
                )
            nc.gpsimd.affine_select(
                out=masks,
                in_=masks,
                compare_op=ALU.is_ge,
                fill=0.0,
                base=-128,
                pattern=[[-256, 2], [1, QB]],
                channel_multiplier=-1,
            )
            bvb_r = bvb_sb.rearrange("p (h d) -> p h d", d=DH)

            def emit_qkv_group(qb2, x1_b, x2_b, qt_b, kind, idx):
                """One fp8-DR psum accumulation group of the qb2 projections.

                kind 'q'/'k': output j-tile idx of Q~/K~; kind 'v': seq
                chunk idx of V. 12 (QK_SETS=3) DR matmuls, each contracting
                2 mt-tiles (256 channels) at 0.5 cyc/row.
                """
                qs2 = slice(qb2 * QB, (qb2 + 1) * QB)
                ps = ps_mm.tile([128, QB], F32, tag="mm")
                if kind in ("q", "k"):
                    w1, w2 = (w_q1, w_q2) if kind == "q" else (w_k1, w_k2)
                    jt = idx
                    js = slice(jt * 128, (jt + 1) * 128)
                    sets = [(w1, x1_b), (w1, x2_b), (w2, x1_b)][:QK_SETS]
                    for si, (w_s, x_s) in enumerate(sets):
                        for m in range(4):
                            nc.tensor.matmul(
                                ps,
                                w_s[:, 2 * m : 2 * m + 2, js],
                                x_s[:, 2 * m : 2 * m + 2, :],
                                start=(si == 0 and m == 0),
                                stop=(si == len(sets) - 1 and m == 3),
                                perf_mode=DR,
                            )
                    if kind == "q":
                        # Q~ = fp8(ps/64 + 8 bq) = fp8(8 Q)
                        nc.vector.tensor_scalar(
                            qt_b[:, jt, :], ps, 1.0 / 64, bq_sb[:, jt : jt + 1],
                            ALU.mult, ALU.add,
                        )
                    else:
                        # K1 = fp8(8K), K2 = fp8(8K - K1): the on-device
                        # residual absorbs K1's rounding exactly. The psum
                        # read must be DVE; the SBUF-only quantize/subtract
                        # run on the otherwise-idle gpsimd so the filler
                        # chain doesn't serialize on the DVE queue
                        s8 = s8p.tile([128, QB], BF16, tag="s8")
                        nc.vector.tensor_scalar(
                            s8, ps, 1.0 / 64, bk_sb[:, jt : jt + 1],
                            ALU.mult, ALU.add,
                        )
                        nc.vector.tensor_copy(kt8[:, jt, 0, qs2], s8)
                        nc.vector.tensor_tensor(
                            kt8[:, jt, 1, qs2], s8, kt8[:, jt, 0, qs2],
                            ALU.subtract,
                        )
                else:
                    kc = idx
                    ks = slice(kc * 128, (kc + 1) * 128)
                    sets = [(x1_b, w_v1), (x2_b, w_v1), (x1_b, w_v2)][:QK_SETS]
                    for si, (x_s, w_s) in enumerate(sets):
                        for m in range(4):
                            nc.tensor.matmul(
                                ps,
                                x_s[:, 2 * m : 2 * m + 2, ks],
                                w_s[:, 2 * m : 2 * m + 2, :],
                                start=(si == 0 and m == 0),
                                stop=(si == len(sets) - 1 and m == 3),
                                perf_mode=DR,
                            )
                    # v_sb = ps + 512 bv = 512 V (bf16); the 1/512 rides
                    # the sel matrix of the reciprocal broadcast
                    nc.vector.tensor_tensor(
                        v_sb[:, qb2 * 4 + kc, :, 0:DH],
                        ps.rearrange("p (h d) -> p h d", d=DH),
                        bvb_r,
                        ALU.add,
                    )

            GROUPS = [("q", i) for i in range(4)] + [("k", i) for i in range(4)] + [
                ("v", i) for i in range(4)
            ]

            def make_proj_group(qb2, ao_b, et, on_act=False):
                qs2 = slice(qb2 * QB, (qb2 + 1) * QB)

                def emit():
                    ps = ps_mm.tile([128, QB], F32, tag="mm")
                    for ct in range(4):
                        nc.tensor.matmul(
                            ps,
                            w_o[:, ct, et * 128 : (et + 1) * 128],
                            ao_b[:, ct, :],
                            start=(ct == 0),
                            stop=(ct == 3),
                        )
                    y_t = yp.tile([128, QB], BF16, tag="y")
                    if on_act:
                        nc.scalar.activation(y_t, ps, AF.Copy)
                    else:
                        nc.vector.tensor_copy(y_t, ps)
                    nc.sync.dma_start(yt[et * 128 : (et + 1) * 128, qs2], y_t)

                return emit

            # all projections are deferred into attn3 (the most ACT-bound
            # phase); their ao blocks stay alive via aop bufs=4
            proj_queue = []

            # q-block 0 projections up front
            region("qkv0")
            qt_blk = qtp.tile([128, 4, QB], FP8, tag="qt")
            for kind, idx in GROUPS:
                emit_qkv_group(0, x1_cur, x2_cur, qt_blk, kind, idx)

            for qb in range(NQB):
                n_kt = (qb + 1) * 4
                last_qb = qb == NQB - 1

                # stage next q-block: x prefetch + Q~ tile; its 12
                # projection groups weave between attention pairs below
                if qb + 1 < NQB:
                    x1_next = x1p.tile([128, 8, QB], FP8, tag="x1")
                    x2_next = x2p.tile([128, 8, QB], FP8, tag="x2")
                    nqs = slice((qb + 1) * QB, (qb + 2) * QB)
                    nc.sync.dma_start(x1_next, x1_r[:, :, nqs])
                    nc.sync.dma_start(x2_next, x2_r[:, :, nqs])
                    qt_next = qtp.tile([128, 4, QB], FP8, tag="qt")
                    next_groups = list(GROUPS)
                else:
                    x1_next = x2_next = qt_next = None
                    next_groups = []
                if qb == 0:
                    # Wo is first needed in attn3; keep it behind qb1's x
                    # prefetch in the load queue
                    wo_r = wot.rearrange("(ct p) e -> p ct e", p=128)
                    nc.sync.dma_start(w_o, wo_r)

                region(f"attn{qb}")
                ao_blk = aop.tile([128, 4, QB], BF16, tag="ao")
                for hp in range(4):
                    # head pair (2hp, 2hp+1) in partitions 0:64 / 64:128 of
                    # j-tile hp; both share one scores psum tile so a
                    # single exp covers the pair
                    filler = []
                    if last_qb:
                        # distribute the 24 deferred proj groups ~evenly
                        quota = (len(proj_queue) + 3 - hp) // (4 - hp)
                        for _ in range(quota):
                            if proj_queue:
                                filler.append(proj_queue.pop(0))
                    else:
                        for _ in range(3):
                            if next_groups:
                                kind, idx = next_groups.pop(0)
                                filler.append(
                                    lambda k=kind, i=idx: emit_qkv_group(
                                        qb + 1, x1_next, x2_next, qt_next, k, i
                                    )
                                )

                    ps_e = ps_o_pool.tile([128, QB], F32, tag="o")
                    ps_o2 = ps_o_pool.tile([128, QB], F32, tag="o")

                    def emit_scores(kt):
                        """Scores + exp + mask for one k-tile; returns the
                        p2 tile. Emitted one kt AHEAD of the attnV consumer
                        so the in-order PE kicks exp(kt+1) before stalling
                        on exp(kt) -> ACT runs exps back-to-back."""
                        kts = slice(kt * 128, (kt + 1) * 128)
                        r = kt - qb * 4
                        live0 = max(r, 0) * 128
                        # one DR matmul per head: groups (K1, K2) refine the
                        # K side; Q~ rides both groups via stride-0 bcast
                        ps_sc = ps_s_pool.tile([128, 2, QB], F32, tag="s")
                        nc.tensor.matmul(
                            ps_sc[:, 0, live0:QB],
                            kt8[0:64, hp, :, kts],
                            qt_blk[0:64, hp, None, live0:QB].to_broadcast(
                                (64, 2, QB - live0)
                            ),
                            start=True,
                            stop=True,
                            perf_mode=DR,
                        )
                        nc.tensor.matmul(
                            ps_sc[:, 1, live0:QB],
                            kt8[64:128, hp, :, kts],
                            qt_blk[64:128, hp, None, live0:QB].to_broadcast(
                                (64, 2, QB - live0)
                            ),
                            start=True,
                            stop=True,
                            perf_mode=DR,
                        )
                        # raw scores are (8K)(8Q) = 64x; exp folds the 1/64
                        p2 = pp.tile([128, 2, QB], BF16, tag="p")
                        nc.scalar.activation(
                            p2[:, :, live0:QB],
                            ps_sc[:, :, live0:QB],
                            AF.Exp,
                            scale=SCALE / 64,
                        )
                        if r >= 0:
                            band = slice(live0, live0 + 128)
                            nc.vector.tensor_tensor(
                                p2[:, :, band],
                                p2[:, :, band],
                                masks[:, 0, None, 128:256].to_broadcast(
                                    (128, 2, 128)
                                ),
                                ALU.mult,
                            )
                        return p2

                    p2_cur = emit_scores(0)
                    for kt in range(n_kt):
                        r = kt - qb * 4
                        live0 = max(r, 0) * 128
                        p2_next = emit_scores(kt + 1) if kt + 1 < n_kt else None
                        # filler lands between scores(kt+1) and attnV(kt):
                        # it overlaps the exp(kt) wait without delaying the
                        # exp(kt+1) kick-off
                        reserve = 0 if (last_qb and hp == 3) else 2
                        if kt % 2 == 1 and kt != n_kt - 1 and len(filler) > reserve:
                            filler.pop(0)()
                        nc.tensor.matmul(
                            ps_e[0 : DH + 1, live0:QB],
                            v_sb[:, kt, 2 * hp, :],
                            p2_cur[:, 0, live0:QB],
                            start=(kt == 0),
                            stop=(kt == n_kt - 1),
                        )
                        nc.tensor.matmul(
                            ps_o2[0 : DH + 1, live0:QB],
                            v_sb[:, kt, 2 * hp + 1, :],
                            p2_cur[:, 1, live0:QB],
                            start=(kt == 0),
                            stop=(kt == n_kt - 1),
                        )
                        p2_cur = p2_next

                    r2 = rp.tile([1, 2, QB], F32R, tag="r2")
                    with nc.allow_low_precision(
                        reason="recip rows feed an fp32r matmul; fp32r"
                        " rounding (~1e-4 rel) is within tolerance"
                    ):
                        nc.vector.reciprocal(r2[:, 0, :], ps_e[DH : DH + 1, :])
                        nc.vector.reciprocal(r2[:, 1, :], ps_o2[DH : DH + 1, :])
                    # one filler group covers the recip latency
                    if filler:
                        filler.pop(0)()
                    # broadcast the two recips across 64 partitions with
                    # tiny ones-row matmuls (sel entries are 1/512,
                    # descaling the 512*V numerator for free)
                    ps_bp = ps_s_pool.tile([128, 2, QB], F32, tag="s")
                    nc.tensor.matmul(
                        ps_bp[0:64, 0, :], sel_sb[:, 0:64], r2[:, 0, :],
                        start=True, stop=True,
                    )
                    nc.tensor.matmul(
                        ps_bp[0:64, 1, :], sel_sb[:, 64:128], r2[:, 1, :],
                        start=True, stop=True,
                    )
                    # bc copies on ACT: at the pair boundary ACT is idle
                    # (next pair's scores haven't landed), while DVE still
                    # has the recips and ao muls queued
                    bc_sb = bcp.tile([128, QB], F32, tag="bcs")
                    nc.vector.tensor_copy(bc_sb[0:64, :], ps_bp[0:64, 0, :])
                    nc.vector.tensor_copy(bc_sb[64:128, :], ps_bp[0:64, 1, :])
                    nc.vector.tensor_mul(
                        ao_blk[0:64, hp, :], ps_e[0:DH, :], bc_sb[0:64, :]
                    )
                    nc.vector.tensor_mul(
                        ao_blk[64:128, hp, :], ps_o2[0:DH, :], bc_sb[64:128, :]
                    )

                    # remaining filler at the pair boundary
                    while filler:
                        filler.pop(0)()

                while next_groups:
                    kind, idx = next_groups.pop(0)
                    emit_qkv_group(qb + 1, x1_next, x2_next, qt_next, kind, idx)
                proj_queue.extend(
                    make_proj_group(qb, ao_blk, et, on_act=last_qb)
                    for et in range(8)
                )
                qt_blk = qt_next

            # drain the last q-block's projection
            region("proj3")
            while proj_queue:
                proj_queue.pop(0)()

    nc.compile()
    return nc


def make_in_maps(x, Wq_w, Wk_w, Wv_w, Wo_w, Wq_b, Wk_b, Wv_b):
    """Per-core host-side sharding, fp8 residual quantization, layout prep."""
    x = np.asarray(x, dtype=np.float32)

    def split8(a, s):
        a = np.ascontiguousarray(a, dtype=np.float32) * np.float32(s)
        a1 = a.astype(E4)
        a2 = (a - a1.astype(np.float32)).astype(E4)
        return a1, a2

    sel = np.full((1, 128), 1.0 / 512, dtype=np.float32)

    in_maps = []
    for c in range(NCORES):
        b, g = divmod(c, 2)
        cols = slice(g * C, (g + 1) * C)
        x1, x2 = split8(x[b].T, 4)
        wq1, wq2 = split8(np.asarray(Wq_w).T[:, cols], 128)
        wk1, wk2 = split8(np.asarray(Wk_w).T[:, cols], 128)
        wv1, wv2 = split8(np.asarray(Wv_w).T[:, cols], 128)
        in_maps.append(
            {
                "x1t": x1,
                "x2t": x2,
                "wq1t": wq1,
                "wq2t": wq2,
                "wk1t": wk1,
                "wk2t": wk2,
                "wv1t": wv1,
                "wv2t": wv2,
                "wot": np.ascontiguousarray(
                    np.asarray(Wo_w)[:, cols].T
                ).astype(ml_dtypes.bfloat16),
                "bq8": np.ascontiguousarray(
                    8.0 * np.asarray(Wq_b)[cols].reshape(C // 128, 128).T
                ).astype(np.float32),
                "bk8": np.ascontiguousarray(
                    8.0 * np.asarray(Wk_b)[cols].reshape(C // 128, 128).T
                ).astype(np.float32),
                "bvb": np.ascontiguousarray(
                    512.0 * np.tile(np.asarray(Wv_b)[cols][None, :], (128, 1))
                ).astype(np.float32),
                "sel": sel,
            }
        )
    return in_maps


_NC_CACHE = {}
last_results = None  # test harness reads profiling info from here


def kernel(x, mask, Wq_w, Wq_b, Wk_w, Wk_b, Wv_w, Wv_b, Wo_w, Wo_b):
    global last_results
    if "nc" not in _NC_CACHE:
        _NC_CACHE["nc"] = build_nc()
    nc = _NC_CACHE["nc"]

    in_maps = make_in_maps(x, Wq_w, Wk_w, Wv_w, Wo_w, Wq_b, Wk_b, Wv_b)
    res = run_bass_kernel_spmd(nc, in_maps, list(range(NCORES)))
    last_results = res

    bo = np.asarray(Wo_b, dtype=np.float32)
    y = np.empty((B, S, D), dtype=np.float32)
    for b in range(B):
        yt = (
            res.results[2 * b]["yt"].astype(np.float32)
            + res.results[2 * b + 1]["yt"].astype(np.float32)
        )
        y[b] = yt.T + bo[None, :]
    return y
